# revision 1
# baseline (speedup 1.0000x reference)
import numpy as np

DIM = 33


def _trilinear_np(LUT, x):
    """Pure-numpy trilinear 3D LUT apply. x: [B,3,H,W], LUT: [3,d,d,d]."""
    dim = DIM
    binsize = 1.0001 / (dim - 1)
    inv = np.float32(1.0 / binsize)
    lut_flat = np.ascontiguousarray(LUT.reshape(3, dim * dim * dim))
    out = np.empty_like(x)
    for i in range(x.shape[0]):
        r, g, b = x[i, 0], x[i, 1], x[i, 2]
        r_s, g_s, b_s = r * inv, g * inv, b * inv
        r_id = np.clip(np.floor(r_s), 0, dim - 2).astype(np.int32)
        g_id = np.clip(np.floor(g_s), 0, dim - 2).astype(np.int32)
        b_id = np.clip(np.floor(b_s), 0, dim - 2).astype(np.int32)
        r_d = r_s - r_id.astype(np.float32)
        g_d = g_s - g_id.astype(np.float32)
        b_d = b_s - b_id.astype(np.float32)
        base = r_id + g_id * dim + b_id * (dim * dim)
        acc = np.zeros((3,) + r.shape, np.float32)
        for db in (0, 1):
            wb = b_d if db else 1.0 - b_d
            for dg in (0, 1):
                wg = g_d if dg else 1.0 - g_d
                for dr in (0, 1):
                    wr = r_d if dr else 1.0 - r_d
                    idx = base + (dr + dg * dim + db * dim * dim)
                    v = lut_flat[:, idx.ravel()].reshape((3,) + r.shape)
                    acc += (wr * wg * wb)[None].astype(np.float32) * v
        out[i] = acc
    return out


def _trilinear_jax_pmap(LUT, x):
    """Data-parallel over batch on 8 NeuronCores; LUT replicated."""
    import jax
    import jax.numpy as jnp

    devs = jax.devices()
    if len(devs) < 8 or x.shape[0] != 8:
        raise RuntimeError("need 8 devices and batch 8")

    dim = DIM
    binsize = 1.0001 / (dim - 1)

    def per_image(lut_flat, img):  # img: [3,H,W]
        r, g, b = img[0], img[1], img[2]
        r_s, g_s, b_s = r / binsize, g / binsize, b / binsize
        r_id = jnp.clip(jnp.floor(r_s).astype(jnp.int32), 0, dim - 2)
        g_id = jnp.clip(jnp.floor(g_s).astype(jnp.int32), 0, dim - 2)
        b_id = jnp.clip(jnp.floor(b_s).astype(jnp.int32), 0, dim - 2)
        r_d = r_s - r_id.astype(img.dtype)
        g_d = g_s - g_id.astype(img.dtype)
        b_d = b_s - b_id.astype(img.dtype)
        base = r_id + g_id * dim + b_id * dim * dim
        acc = jnp.zeros((3,) + r.shape, img.dtype)
        for db in (0, 1):
            wb = b_d if db else (1.0 - b_d)
            for dg in (0, 1):
                wg = g_d if dg else (1.0 - g_d)
                for dr in (0, 1):
                    wr = r_d if dr else (1.0 - r_d)
                    idx = base + (dr + dg * dim + db * dim * dim)
                    v = lut_flat[:, idx]  # [3,H,W]
                    acc = acc + (wr * wg * wb)[None] * v
        return acc

    f = jax.pmap(per_image, in_axes=(None, 0), devices=devs[:8])
    y = f(LUT.reshape(3, dim * dim * dim), x)
    return np.asarray(y)


def kernel(LUT=None, x=None, **kwargs):
    LUT = np.asarray(LUT, dtype=np.float32)
    x = np.asarray(x, dtype=np.float32)
    try:
        return _trilinear_jax_pmap(LUT, x)
    except Exception:
        return _trilinear_np(LUT, x)

